# revision 13
# baseline (speedup 1.0000x reference)
"""Causal self-attention (B=4, T=2048, DIM=1024, H=8, D=128) on 8 trn2 cores.

Sharding: core i handles batch b = i//2, head-group g = i%2 (4 heads each).
Per-core: fused QKV (fp32r matmuls), per-head RMSnorm + RoPE, causal
attention in scores^T layout (softmax over the partition axis: denominator
via M=1 PE matmuls), lambda-mix of V with ve (lambdas folded host-side),
c_proj partial. Host sums the two head-group partials per batch.

All big matmuls run as float32r (TF32) at 1 cycle/row; probs/V/c_proj use
bf16 operands (fp32 PSUM accumulate).
"""
import sys

sys.path.insert(0, "/opt/trn_rl_repo")

from contextlib import ExitStack

import numpy as np
import ml_dtypes

import concourse.bass as bass  # noqa: F401
import concourse.mybir as mybir
import concourse.tile as tile
from concourse import bacc
from concourse.bass_utils import run_bass_kernel_spmd

B, T, DIM, H, D = 4, 2048, 1024, 8, 128
HG = 2              # head-groups (tensor-parallel factor per batch)
HPG = H // HG       # heads per core
CS = 512            # t-chunk size (PSUM fp32 bank = 512 cols)
NCH = T // CS       # 4 chunks
KT = T // 128       # 16 tk tiles
KD = DIM // 128     # 8 contraction tiles
FQK = HPG * 2 * 128  # 1024 qk feature cols per core
FV = HPG * 128       # 512 v cols per core
F32 = mybir.dt.float32
R32 = mybir.dt.float32r
BF16 = mybir.dt.bfloat16
EPS = float(np.finfo(np.float32).eps)
SCALE = float(D ** -0.5)
MUL = mybir.AluOpType.mult
ADD = mybir.AluOpType.add
SUB = mybir.AluOpType.subtract

_cache = {}


def _rope_tables():
    freq = (1.0 / 1024.0) ** np.linspace(0.0, 1.0, D // 4, dtype=np.float64)
    freq = np.concatenate([freq, np.zeros(D // 4)])
    theta = np.arange(T, dtype=np.float64)[:, None] * freq[None, :]  # [T, 64]
    cos = np.cos(theta).astype(np.float32).T.copy()  # [64, T]
    sin = np.sin(theta).astype(np.float32).T.copy()
    return cos, sin


def _masks():
    # mask_r[i, j] = 1 if j - i >= 128*r  (tk tile at offset r*128 inside a
    # 512-wide tq chunk); concatenated along free dim -> [128, 4*512]
    i = np.arange(128)[:, None]
    j = np.arange(CS)[None, :]
    tiles = [(j - i >= 128 * r).astype(np.float32) for r in range(4)]
    return np.concatenate(tiles, axis=1).astype(ml_dtypes.bfloat16)


def _phase_a(nc, tc, ctx, dram, P):
    """QKV projection, v-mix, RMSnorm stats, RoPE, normalize into qkT."""
    wqk_pool = ctx.enter_context(tc.tile_pool(name="w", bufs=KD))
    wv_pool = ctx.enter_context(tc.tile_pool(name="wvp", bufs=KD))
    xt_pool = ctx.enter_context(tc.tile_pool(name="xt", bufs=10))
    ve_pool = ctx.enter_context(tc.tile_pool(name="vep", bufs=2))
    raw_pool = ctx.enter_context(tc.tile_pool(name="raw", bufs=3))
    rt_pool = ctx.enter_context(tc.tile_pool(name="rtmp", bufs=4))
    rop_pool = ctx.enter_context(tc.tile_pool(name="rop", bufs=10))
    tab_pool = ctx.enter_context(tc.tile_pool(name="tab", bufs=2))
    ms_pool = ctx.enter_context(tc.tile_pool(name="ms", bufs=2))
    pa_pool = ctx.enter_context(tc.tile_pool(name="pa", bufs=4, space="PSUM"))
    pss_pool = ctx.enter_context(tc.tile_pool(name="pss", bufs=2, space="PSUM"))
    pbc_pool = ctx.enter_context(tc.tile_pool(name="pbc", bufs=2, space="PSUM"))

    w_qk = [wqk_pool.tile([128, FQK], R32, tag="wqk", name=f"wqk{i}") for i in range(KD)]
    w_v = [wv_pool.tile([128, FV], R32, tag="wv", name=f"wv{i}") for i in range(KD)]
    for kd in range(KD):
        ksl = slice(kd * 128, (kd + 1) * 128)
        nc.sync.dma_start(w_qk[kd][:], dram["wqk"].ap()[ksl, :].bitcast(R32))
        nc.sync.dma_start(w_v[kd][:], dram["wv"].ap()[ksl, :].bitcast(R32))

    for c in range(NCH):
        csl = slice(c * CS, (c + 1) * CS)
        xts = []
        for kd in range(KD):
            xt_t = xt_pool.tile([128, CS], R32, tag="xt", name=f"xt{c}_{kd}")
            nc.sync.dma_start(
                xt_t[:], dram["xt"].ap()[kd * 128:(kd + 1) * 128, csl].bitcast(R32)
            )
            xts.append(xt_t)

        # tables duplicated across both partition halves (DVE requires equal
        # base partitions when both tensor_tensor inputs are in SBUF)
        cos_t = tab_pool.tile([128, CS], F32, tag="cos")
        sin_t = tab_pool.tile([128, CS], F32, tag="sin")
        nc.sync.dma_start(cos_t[0:64, :], dram["cos"].ap()[:, csl])
        nc.sync.dma_start(cos_t[64:128, :], dram["cos"].ap()[:, csl])
        nc.sync.dma_start(sin_t[0:64, :], dram["sin"].ap()[:, csl])
        nc.sync.dma_start(sin_t[64:128, :], dram["sin"].ap()[:, csl])

        # v for this chunk's 4 token sub-tiles
        for sub in range(4):
            ti = c * 4 + sub
            pv = pa_pool.tile([128, FV], F32, tag="pa")
            for kd in range(KD):
                nc.tensor.matmul(
                    pv[:], xts[kd][:, sub * 128:(sub + 1) * 128], w_v[kd][:],
                    start=(kd == 0), stop=(kd == KD - 1),
                )
            ve_t = ve_pool.tile([128, FV], F32, tag="ve")
            nc.sync.dma_start(ve_t[:], dram["ve"].ap()[ti * 128:(ti + 1) * 128, :])
            nc.vector.tensor_tensor(P["v_bf"][ti][:], pv[:], ve_t[:], ADD)

        # q/k per head: project, sumsq, rope
        rops = []
        rstds = []
        for h in range(HPG):
            for qi in range(2):
                f0 = h * 256 + qi * 128
                pqk = pa_pool.tile([128, CS], F32, tag="pa")
                for kd in range(KD):
                    nc.tensor.matmul(
                        pqk[:], w_qk[kd][:, f0:f0 + 128], xts[kd][:],
                        start=(kd == 0), stop=(kd == KD - 1),
                    )
                raw = raw_pool.tile([128, CS], F32, tag="raw")
                nc.scalar.copy(raw[:], pqk[:])
                # mean of squares over the 128 head dims (partition axis):
                # Square(raw/sqrt(128)) summed by a ones matmul = mean
                sq = raw_pool.tile([128, CS], R32, tag="sq")
                nc.scalar.activation(
                    sq[:], raw[:], mybir.ActivationFunctionType.Square, scale=SCALE
                )
                ssps = pss_pool.tile([1, CS], F32, tag="ss")
                nc.tensor.matmul(ssps[:], P["ones_r"][:], sq[:], start=True, stop=True)
                ms_r = ms_pool.tile([1, CS], F32, tag="ms", bufs=4)
                nc.vector.tensor_scalar_add(ms_r[:], ssps[:], EPS)
                inv_r = ms_pool.tile([1, CS], F32, tag="inv", bufs=4)
                nc.vector.reciprocal(inv_r[:], ms_r[:])
                rstd = ms_pool.tile([1, CS], R32, tag="rstd", bufs=8,
                                    name=f"rstd{c}_{2 * h + qi}")
                nc.scalar.sqrt(rstd[:], inv_r[:])
                rstds.append(rstd)
                # rope: rows 0:64 = x1*c + x2*s ; rows 64:128 = x2*c - x1*s
                t_c1 = rt_pool.tile([64, CS], F32, tag="rt")
                t_s2 = rt_pool.tile([64, CS], F32, tag="rt")
                t_c2 = rt_pool.tile([64, CS], F32, tag="rt")
                t_s1 = rt_pool.tile([64, CS], F32, tag="rt")
                nc.vector.tensor_tensor(t_c1[:], raw[0:64, :], cos_t[0:64, :], MUL)
                nc.vector.tensor_tensor(t_s2[:], raw[64:128, :], sin_t[64:128, :], MUL)
                nc.vector.tensor_tensor(t_c2[:], raw[64:128, :], cos_t[64:128, :], MUL)
                nc.vector.tensor_tensor(t_s1[:], raw[0:64, :], sin_t[0:64, :], MUL)
                rop = rop_pool.tile([128, CS], F32, tag="rop")
                nc.vector.tensor_tensor(rop[0:64, :], t_c1[:], t_s2[:], ADD)
                nc.vector.tensor_tensor(rop[64:128, :], t_c2[:], t_s1[:], SUB)
                rops.append(rop)

        # normalize rope outputs into qkT
        for row in range(8):
            pbc = pbc_pool.tile([128, CS], F32, tag="bc")
            nc.tensor.matmul(
                pbc[:], P["ones1_r"][:], rstds[row][:], start=True, stop=True
            )
            nc.vector.tensor_tensor(P["qkT"][row][:, csl], rops[row][:], pbc[:], MUL)


def _phase_b(nc, tc, ctx, P):
    """Causal attention per head, scores^T layout."""
    ex_pool = ctx.enter_context(tc.tile_pool(name="exp", bufs=KT))
    sm_pool = ctx.enter_context(tc.tile_pool(name="sm", bufs=3))
    rb_pool = ctx.enter_context(tc.tile_pool(name="rb", bufs=2))
    pb_pool = ctx.enter_context(tc.tile_pool(name="pb", bufs=3, space="PSUM"))
    py_pool = ctx.enter_context(tc.tile_pool(name="py", bufs=2, space="PSUM"))
    pd_pool = ctx.enter_context(tc.tile_pool(name="pd", bufs=2, space="PSUM"))
    pn_pool = ctx.enter_context(tc.tile_pool(name="pn", bufs=1, space="PSUM"))

    for h in range(HPG):
        qh, kh = P["qkT"][2 * h], P["qkT"][2 * h + 1]
        for c in range(NCH):
            csl = slice(c * CS, (c + 1) * CS)
            nkt = 4 * (c + 1)
            exs = []
            for kt in range(nkt):
                ps = pb_pool.tile([128, CS], F32, tag="s")
                nc.tensor.matmul(
                    ps[:], kh[:, kt * 128:(kt + 1) * 128], qh[:, csl],
                    start=True, stop=True,
                )
                ex = ex_pool.tile([128, CS], BF16, tag="ex")
                nc.scalar.activation(
                    ex[:], ps[:], mybir.ActivationFunctionType.Exp, scale=SCALE
                )
                r = kt - 4 * c
                if r >= 0:
                    nc.vector.tensor_tensor(
                        ex[:], ex[:], P["mask_t"][:, r * CS:(r + 1) * CS], MUL
                    )
                exs.append(ex)
            yac = py_pool.tile([128, CS], F32, tag="y")
            den = pd_pool.tile([1, CS], F32, tag="d")
            for kt in range(nkt):
                nc.tensor.matmul(
                    yac[:], P["v_bf"][kt][:, h * 128:(h + 1) * 128], exs[kt][:],
                    start=(kt == 0), stop=(kt == nkt - 1),
                )
            for kt in range(nkt):
                nc.tensor.matmul(
                    den[:], P["ones_b"][:], exs[kt][:],
                    start=(kt == 0), stop=(kt == nkt - 1),
                )
            rcp = sm_pool.tile([1, CS], R32, tag="rcp")
            nc.vector.reciprocal(rcp[:], den[:])
            pnb = pn_pool.tile([128, CS], F32, tag="nb")
            nc.tensor.matmul(pnb[:], P["ones1_r"][:], rcp[:], start=True, stop=True)
            rbc = rb_pool.tile([128, CS], F32, tag="rb")
            nc.scalar.copy(rbc[:], pnb[:])
            nc.vector.tensor_tensor(P["yT"][h][:, csl], yac[:], rbc[:], MUL)


def _phase_c(nc, tc, ctx, dram, P):
    """c_proj partial: oT[m, t] = sum_j cwT[j, m] * yT[j, t]."""
    cw_pool = ctx.enter_context(tc.tile_pool(name="cwp", bufs=HPG))
    os_pool = ctx.enter_context(tc.tile_pool(name="os", bufs=4))
    pc_pool = ctx.enter_context(tc.tile_pool(name="pc", bufs=4, space="PSUM"))

    cwt = [cw_pool.tile([128, DIM], BF16, tag="cw", name=f"cw{i}") for i in range(HPG)]
    for j in range(HPG):
        nc.sync.dma_start(cwt[j][:], dram["cw"].ap()[j * 128:(j + 1) * 128, :])
    for m in range(KD):
        msl = slice(m * 128, (m + 1) * 128)
        for c in range(NCH):
            csl = slice(c * CS, (c + 1) * CS)
            po = pc_pool.tile([128, CS], F32, tag="pc")
            for j in range(HPG):
                nc.tensor.matmul(
                    po[:], cwt[j][:, msl], P["yT"][j][:, csl],
                    start=(j == 0), stop=(j == HPG - 1),
                )
            so = os_pool.tile([128, CS], F32, tag="os")
            nc.scalar.copy(so[:], po[:])
            nc.sync.dma_start(dram["ot"].ap()[msl, csl], so[:])


def _build_program():
    nc = bacc.Bacc("TRN2", target_bir_lowering=False, debug=False, num_devices=B * HG)

    dram = {
        "xt": nc.dram_tensor("xt", [DIM, T], F32, kind="ExternalInput"),
        "wqk": nc.dram_tensor("wqk", [DIM, FQK], F32, kind="ExternalInput"),
        "wv": nc.dram_tensor("wv", [DIM, FV], F32, kind="ExternalInput"),
        "ve": nc.dram_tensor("ve", [T, FV], F32, kind="ExternalInput"),
        "cw": nc.dram_tensor("cw", [FV, DIM], BF16, kind="ExternalInput"),
        "cos": nc.dram_tensor("cos", [64, T], F32, kind="ExternalInput"),
        "sin": nc.dram_tensor("sin", [64, T], F32, kind="ExternalInput"),
        "mask": nc.dram_tensor("mask", [128, 4 * CS], BF16, kind="ExternalInput"),
        "ones": nc.dram_tensor("ones", [128, 1], F32, kind="ExternalInput"),
        "ones1": nc.dram_tensor("ones1", [1, 128], F32, kind="ExternalInput"),
        "onesbf": nc.dram_tensor("onesbf", [128, 1], BF16, kind="ExternalInput"),
        "ot": nc.dram_tensor("ot", [DIM, T], F32, kind="ExternalOutput"),
    }

    with ExitStack() as top:
        top.enter_context(nc.allow_low_precision(reason="bf16 probs/V/c_proj by design"))
        tc = top.enter_context(tile.TileContext(nc))
        qk_pool = top.enter_context(tc.tile_pool(name="qk", bufs=2 * HPG))
        v_pool = top.enter_context(tc.tile_pool(name="vbf", bufs=KT))
        c_pool = top.enter_context(tc.tile_pool(name="const", bufs=1))

        P = {
            "qkT": [qk_pool.tile([128, T], BF16, tag="qk", name=f"qkT{i}")
                    for i in range(2 * HPG)],
            "v_bf": [v_pool.tile([128, FV], BF16, tag="v", name=f"vbf{i}")
                     for i in range(KT)],
            "ones_r": c_pool.tile([128, 1], R32, tag="ones", name="ones_r"),
            "ones1_r": c_pool.tile([1, 128], R32, tag="ones1", name="ones1_r"),
            "ones_b": c_pool.tile([128, 1], BF16, tag="onesbf", name="ones_b"),
        }
        nc.sync.dma_start(P["ones_r"][:], dram["ones"].ap().bitcast(R32))
        nc.sync.dma_start(P["ones1_r"][:], dram["ones1"].ap().bitcast(R32))
        nc.sync.dma_start(P["ones_b"][:], dram["onesbf"].ap())

        with ExitStack() as ctx_a:
            _phase_a(nc, tc, ctx_a, dram, P)
        with ExitStack() as ctx_bc:
            y_pool = ctx_bc.enter_context(tc.tile_pool(name="yt", bufs=HPG))
            m_pool = ctx_bc.enter_context(tc.tile_pool(name="maskp", bufs=1))
            P["yT"] = [y_pool.tile([128, T], BF16, tag="y", name=f"yT{i}")
                       for i in range(HPG)]
            P["mask_t"] = m_pool.tile([128, 4 * CS], BF16, tag="mask", name="mask_t")
            nc.sync.dma_start(P["mask_t"][:], dram["mask"].ap())
            with ExitStack() as ctx_b:
                _phase_b(nc, tc, ctx_b, P)
            with ExitStack() as ctx_c:
                _phase_c(nc, tc, ctx_c, dram, P)

    nc.compile()
    return nc


def _prep_inputs(x, ve, qkv_w, lambdas, c_proj_w):
    cos, sin = _rope_tables()
    mask = _masks()
    ones = np.ones((128, 1), np.float32)
    ones1 = np.ones((1, 128), np.float32)
    onesbf = np.ones((128, 1), ml_dtypes.bfloat16)
    qw, kw, vw = qkv_w[0], qkv_w[1], qkv_w[2]

    in_maps = []
    for core in range(B * HG):
        b, g = core // HG, core % HG
        heads = range(g * HPG, (g + 1) * HPG)
        rows = np.concatenate(
            [np.concatenate([qw[h * D:(h + 1) * D], kw[h * D:(h + 1) * D]])
             for h in heads]
        )                                    # [1024, DIM]
        vcols = slice(g * HPG * D, (g + 1) * HPG * D)
        in_maps.append({
            "xt": np.ascontiguousarray(x[b].T),
            "wqk": np.ascontiguousarray(rows.T),
            "wv": np.ascontiguousarray((lambdas[0] * vw[vcols]).T),
            "ve": np.ascontiguousarray(lambdas[1] * ve[b][:, vcols]),
            "cw": np.ascontiguousarray(c_proj_w[:, vcols].T).astype(ml_dtypes.bfloat16),
            "cos": cos,
            "sin": sin,
            "mask": mask,
            "ones": ones,
            "ones1": ones1,
            "onesbf": onesbf,
        })
    return in_maps


def kernel(x, ve, qkv_w, lambdas, c_proj_w):
    x = np.asarray(x, np.float32)
    ve = np.asarray(ve, np.float32)
    qkv_w = np.asarray(qkv_w, np.float32).reshape(3, H * D, DIM)
    lambdas = np.asarray(lambdas, np.float32)
    c_proj_w = np.asarray(c_proj_w, np.float32)

    if "nc" not in _cache:
        _cache["nc"] = _build_program()
    nc = _cache["nc"]

    in_maps = _prep_inputs(x, ve, qkv_w, lambdas, c_proj_w)
    res = run_bass_kernel_spmd(nc, in_maps, list(range(B * HG))).results

    out = np.empty((B, T, DIM), np.float32)
    for b in range(B):
        acc = res[HG * b]["ot"].astype(np.float32)
        for g in range(1, HG):
            acc = acc + res[HG * b + g]["ot"]
        out[b] = acc.T
    return out
